# revision 6
# baseline (speedup 1.0000x reference)
"""Trainium2 Bass kernel for nn_MPDWConv (B=8, E=256, H=W=128).

Sharding: data-parallel over batch - each of the 8 NeuronCores processes one
full image.

Per-core pipeline (channel-major [c, h, w], guard-padded tiles so every
depthwise tap is a pure free-dim offset):
  1. x arrives host-padded as bf16 [2, 128, 130, 130] (pad 1, zero guards),
     DMA'd straight into SBUF - no on-device spreading.
  2. Stage-1 3x3 depthwise conv into guard-padded x0 tiles
     (block0 [128,134,134] pad 3, block1 [128,140,140] pad 6), engine mix
     per S1_PAT (quad granularity, 4 windows x 1 block each):
       'p' PE diag-matmuls + Act PSUM copy, 'v' DVE hybrid
       (tensor_scalar 4x + tensor_tensor 2x), 'g' Pool tensor_scalar/
       scalar_tensor_tensor chains, 'A' Act scaled copies + DVE adds.
  3. Branch convs per 4-window quad: 5x5 dil3 (ch 128..255, shared w2) on
     PE as 25 PSUM-accumulated diag matmuls (BR5_PAT), 3x3 dil3
     (ch 64..127) as DVE/Act hybrid chains (BR3_PAT); x0[0:64] passthrough
     copied into xc via SBUF->SBUF DMA.
  4. 1x1 pointwise conv: 2 K=128 bf16 GEMMs per (window, out-block);
     PSUM -> bf16 staging copy (PWO_PAT engine) -> DMA out. b_pw and the
     f32 cast are applied on the host.
"""

import os as _os

import numpy as np

B, E, H, W = 8, 256, 128, 128
T9 = [(a, b) for a in range(3) for b in range(3)]
T25 = [(a, b) for a in range(5) for b in range(5)]

# Engine patterns (tuning knobs). S1_PAT: 16 chars, order
# q0b1,q0b0,q1b1,q1b0,... (block1 first). BR5/BR3/PWO: 8 chars, one per
# branch quad.
S1_PAT = _os.environ.get("S1_PAT", "ppppvvvvvvvvvvvv")
BR5_PAT = _os.environ.get("BR5_PAT", "pppppppp")
BR3_PAT = _os.environ.get("BR3_PAT", "AAGGGGGG")
PWO_PAT = _os.environ.get("PWO_PAT", "aaaaaaaa")

_CACHE = {}


def _split_excess_waits(nc, mybir):
    """Walrus legalization: TRN2 instructions encode at most ONE sync wait
    (two for EventSemaphore). Tile attaches multi-wait sync_info; move the
    excess onto same-engine NoOp prefixes."""
    n_created = 0
    for fn in nc.m.functions:
        for blk in fn.blocks:
            insts = list(blk.instructions)
            out = []
            changed = False
            for inst in insts:
                si = getattr(inst, "sync_info", None)
                cap = 2 if isinstance(inst, mybir.InstEventSemaphore) else 1
                if si is not None and si.on_wait is not None \
                        and len(si.on_wait) > cap:
                    waits = list(si.on_wait)
                    extra, keep = waits[:-cap], waits[-cap:]
                    for w in extra:
                        n_created += 1
                        nop = mybir.InstNoOp(
                            name=f"I-waitsplit-{n_created}",
                            engine=inst.engine)
                        nop.sync_info = mybir.SyncInfo(
                            on_wait=[w], on_update=[])
                        out.append(nop)
                    inst.sync_info = mybir.SyncInfo(
                        on_wait=keep, on_update=list(si.on_update))
                    changed = True
                out.append(inst)
            if changed:
                blk.instructions = out
    return n_created


def _build_nc():
    import concourse.bass as bass
    import concourse.mybir as mybir
    from concourse import tile

    bf16 = mybir.dt.bfloat16
    f32 = mybir.dt.float32
    mult, add = mybir.AluOpType.mult, mybir.AluOpType.add
    IDENT = mybir.ActivationFunctionType.Identity
    COPY = mybir.ActivationFunctionType.Copy

    nc = bass.Bass(trn_type="TRN2")

    # ---- DRAM parameters ----
    xb = nc.dram_tensor("xb", [2, 128, 130, 130], bf16, kind="ExternalInput")
    d0 = nc.dram_tensor("d0", [2, 128, 9 * 128], bf16, kind="ExternalInput")
    d1 = nc.dram_tensor("d1", [128, 9 * 128], bf16, kind="ExternalInput")
    d2 = nc.dram_tensor("d2", [128, 25 * 128], bf16, kind="ExternalInput")
    wpw = nc.dram_tensor("wpw", [2, 2, 128, 128], bf16, kind="ExternalInput")
    k0s = nc.dram_tensor("k0s", [2, 128, 9], f32, kind="ExternalInput")
    k1s = nc.dram_tensor("k1s", [128, 9], f32, kind="ExternalInput")
    b0v = nc.dram_tensor("b0v", [2, 128, 1], f32, kind="ExternalInput")
    b1v = nc.dram_tensor("b1v", [128, 1], f32, kind="ExternalInput")
    b2v = nc.dram_tensor("b2v", [128, 1], f32, kind="ExternalInput")
    y = nc.dram_tensor("y", [E, H, W], bf16, kind="ExternalOutput")

    y_ap = y.ap()

    with tile.TileContext(nc) as tc:
        with (
            tc.tile_pool(name="const", bufs=1) as cpool,
            tc.tile_pool(name="xin", bufs=1) as xpool,
            tc.tile_pool(name="x0", bufs=1) as x0pool,
            tc.tile_pool(name="xc0", bufs=2) as xc0pool,
            tc.tile_pool(name="xc1", bufs=2) as xc1pool,
            tc.tile_pool(name="acc", bufs=1) as accpool,
            tc.tile_pool(name="tmp", bufs=2) as tmppool,
            tc.tile_pool(name="ys", bufs=2) as yspool,
            tc.tile_pool(name="ps_s1", bufs=2, space="PSUM") as ps1pool,
            tc.tile_pool(name="ps_br", bufs=2, space="PSUM") as psbrpool,
            tc.tile_pool(name="ps_pw", bufs=2, space="PSUM") as pspwpool,
        ):
            # ---- constants into SBUF (gpsimd SWDGE queue) ----
            d0t = []
            for blk in range(2):
                t = cpool.tile([128, 9 * 128], bf16, tag=f"d0_{blk}")
                nc.gpsimd.dma_start(out=t[:], in_=d0.ap()[blk])
                d0t.append(t)
            d1t = cpool.tile([128, 9 * 128], bf16, tag="d1")
            nc.gpsimd.dma_start(out=d1t[:], in_=d1.ap())
            d2t = cpool.tile([128, 25 * 128], bf16, tag="d2")
            nc.gpsimd.dma_start(out=d2t[:], in_=d2.ap())
            wpwt = [[None, None], [None, None]]
            for k in range(2):
                for ob in range(2):
                    t = cpool.tile([128, 128], bf16, tag=f"wpw_{k}{ob}")
                    nc.gpsimd.dma_start(out=t[:], in_=wpw.ap()[k, ob])
                    wpwt[k][ob] = t
            k0t = []
            for blk in range(2):
                t = cpool.tile([128, 9], f32, tag=f"k0_{blk}")
                nc.gpsimd.dma_start(out=t[:], in_=k0s.ap()[blk])
                k0t.append(t)
            k1t = cpool.tile([128, 9], f32, tag="k1")
            nc.gpsimd.dma_start(out=k1t[:], in_=k1s.ap())
            b0t = []
            for blk in range(2):
                t = cpool.tile([128, 1], f32, tag=f"b0_{blk}")
                nc.gpsimd.dma_start(out=t[:], in_=b0v.ap()[blk])
                b0t.append(t)
            b1t = cpool.tile([128, 1], f32, tag="b1")
            nc.gpsimd.dma_start(out=b1t[:], in_=b1v.ap())
            b2t = cpool.tile([128, 1], f32, tag="b2")
            nc.gpsimd.dma_start(out=b2t[:], in_=b2v.ap())

            # Pre-touch scalar tiles on each consumer engine so steady-state
            # ops don't each carry a DMA-lane sync wait.
            scrA = cpool.tile([128, 9], f32, tag="scrA")
            scrV = cpool.tile([128, 9], f32, tag="scrV")
            for t in (k0t[0], k0t[1], k1t):
                nc.scalar.activation(out=scrA[:], in_=t[:], func=COPY)
                nc.vector.tensor_copy(scrV[:], t[:])
            for t in (b0t[0], b0t[1], b1t, b2t):
                nc.scalar.activation(out=scrA[:, 0:1], in_=t[:], func=COPY)
                nc.vector.tensor_copy(scrV[:, 0:1], t[:])

            # ---- x0 tiles + halo memsets ----
            x0b0 = x0pool.tile([128, 134, 134], bf16, tag="x0b0")
            x0b1 = x0pool.tile([128, 140, 140], bf16, tag="x0b1")
            for t, p, n in ((x0b0, 3, 134), (x0b1, 6, 140)):
                nc.vector.memset(t[:, 0:p, :], 0.0)
                nc.vector.memset(t[:, n - p:n, :], 0.0)
                nc.vector.memset(t[:, p:n - p, 0:p], 0.0)
                nc.vector.memset(t[:, p:n - p, n - p:n], 0.0)

            # ---- input DMAs (4 row-chunks per block, block1 first) ----
            xpt = []
            for blk in range(2):
                t = xpool.tile([128, 130, 130], bf16, tag=f"x_{blk}")
                xpt.append(t)
            chunks = [(0, 33), (33, 66), (66, 99), (99, 130)]
            for r0, r1 in chunks:
                for blk in (1, 0):
                    nc.sync.dma_start(
                        out=xpt[blk][:, r0:r1, :],
                        in_=xb.ap()[blk, :, r0:r1, :])

            # ---------- emission helpers ----------
            def s1_quad_pe(q, blk):
                """4 windows of stage-1 as diag matmuls + Act copies."""
                xt, x0t, pad = ((xpt[0], x0b0, 3) if blk == 0
                                else (xpt[1], x0b1, 6))
                for i in range(4):
                    r0 = 16 * q + 4 * i
                    ps = ps1pool.tile([128, 4, 128], f32, tag="s1")
                    for t, (ty, tx) in enumerate(T9):
                        nc.tensor.matmul(
                            ps[:],
                            lhsT=d0t[blk][:, t * 128:(t + 1) * 128],
                            rhs=xt[:, r0 + ty: r0 + ty + 4, tx: tx + 128],
                            start=(t == 0), stop=(t == 8))
                    nc.scalar.activation(
                        out=x0t[:, pad + r0: pad + r0 + 4,
                                pad: pad + 128],
                        in_=ps[:], func=IDENT, bias=b0t[blk][:], scale=1.0)

            def s1_quad_chain(q, blk, ch):
                """16-row MAC chain: 'v' all-DVE hybrid, 'A' Act copies +
                DVE adds, 'G' Act copies + Pool adds."""
                xt, x0t, pad = ((xpt[0], x0b0, 3) if blk == 0
                                else (xpt[1], x0b1, 6))
                r0 = 16 * q
                out_v = x0t[:, pad + r0: pad + r0 + 16, pad: pad + 128]
                adder = nc.gpsimd if ch == "G" else nc.vector
                acc = accpool.tile([128, 2048], bf16,
                                   tag="a_g" if ch == "G" else "a_v")
                accv = acc[:].rearrange("p (r c) -> p r c", r=16)
                for t, (ty, tx) in enumerate(T9):
                    src = xt[:, r0 + ty: r0 + ty + 16, tx: tx + 128]
                    if t == 0:
                        if ch == "v":
                            nc.vector.tensor_scalar(
                                out=accv, in0=src,
                                scalar1=k0t[blk][:, 0:1],
                                scalar2=b0t[blk][:], op0=mult, op1=add)
                        else:
                            nc.scalar.activation(
                                out=accv, in_=src, func=IDENT,
                                bias=b0t[blk][:], scale=k0t[blk][:, 0:1])
                    else:
                        tm = tmppool.tile([128, 2048], bf16,
                                          tag="t_v" if ch == "v" else "t_a")
                        tmv = tm[:].rearrange("p (r c) -> p r c", r=16)
                        if ch == "v":
                            nc.vector.tensor_scalar(
                                out=tmv, in0=src,
                                scalar1=k0t[blk][:, t: t + 1],
                                scalar2=0.0, op0=mult, op1=add)
                        else:
                            nc.scalar.activation(
                                out=tmv, in_=src, func=COPY,
                                scale=k0t[blk][:, t: t + 1])
                        adder.tensor_tensor(
                            out=(out_v if t == 8 else accv),
                            in0=accv, in1=tmv, op=add)

            def s1_quad(q, blk, ch):
                if ch == "p":
                    s1_quad_pe(q, blk)
                else:
                    s1_quad_chain(q, blk, ch)

            def br5_win_pe(w, xc1q, slot):
                ps = psbrpool.tile([128, 4, 128], f32, tag="br5")
                r0 = 4 * w
                for t, (ty, tx) in enumerate(T25):
                    nc.tensor.matmul(
                        ps[:],
                        lhsT=d2t[:, t * 128:(t + 1) * 128],
                        rhs=x0b1[:, r0 + 3 * ty: r0 + 3 * ty + 4,
                                 3 * tx: 3 * tx + 128],
                        start=(t == 0), stop=(t == 24))
                nc.scalar.activation(
                    out=xc1q[:, 4 * slot: 4 * slot + 4, :],
                    in_=ps[:], func=IDENT, bias=b2t[:], scale=1.0)

            def br3_quad_pe(bq, xc0q):
                r0 = 16 * bq
                for i in range(4):
                    ps = psbrpool.tile([128, 4, 128], f32, tag="br3")
                    for t, (ty, tx) in enumerate(T9):
                        nc.tensor.matmul(
                            ps[:],
                            lhsT=d1t[:, t * 128:(t + 1) * 128],
                            rhs=x0b0[:, r0 + 4 * i + 3 * ty:
                                     r0 + 4 * i + 3 * ty + 4,
                                     3 * tx: 3 * tx + 128],
                            start=(t == 0), stop=(t == 8))
                    nc.scalar.activation(
                        out=xc0q[64:128, 4 * i: 4 * i + 4, :],
                        in_=ps[64:128], func=IDENT,
                        bias=b1t[64:128], scale=1.0)

            def br3_quad_hybrid(bq, xc0q, ch):
                """3x3 dil3 on ch 64..127: 'v' DVE hybrid, 'A' Act+DVE,
                'G' Act+Pool."""
                r0 = 16 * bq
                out_v = xc0q[64:128, :, :]
                adder = nc.gpsimd if ch == "G" else nc.vector
                acc = accpool.tile([128, 2048], bf16,
                                   tag="a_g" if ch == "G" else "a_v")
                accv = acc[64:128].rearrange("p (r c) -> p r c", r=16)
                for t, (ty, tx) in enumerate(T9):
                    src = x0b0[64:128, r0 + 3 * ty: r0 + 3 * ty + 16,
                               3 * tx: 3 * tx + 128]
                    if t == 0:
                        if ch == "v":
                            nc.vector.tensor_scalar(
                                out=accv, in0=src,
                                scalar1=k1t[64:128, 0:1],
                                scalar2=b1t[64:128], op0=mult, op1=add)
                        else:
                            nc.scalar.activation(
                                out=accv, in_=src, func=IDENT,
                                bias=b1t[64:128], scale=k1t[64:128, 0:1])
                    else:
                        tm = tmppool.tile([128, 2048], bf16,
                                          tag="t_v" if ch == "v" else "t_a")
                        tmv = tm[64:128].rearrange("p (r c) -> p r c", r=16)
                        if ch == "v":
                            nc.vector.tensor_scalar(
                                out=tmv, in0=src,
                                scalar1=k1t[64:128, t: t + 1],
                                scalar2=0.0, op0=mult, op1=add)
                        else:
                            nc.scalar.activation(
                                out=tmv, in_=src, func=COPY,
                                scale=k1t[64:128, t: t + 1])
                        adder.tensor_tensor(
                            out=(out_v if t == 8 else accv),
                            in0=accv, in1=tmv, op=add)

            def pw_win(w, xc0q, xc1q, slot, ch):
                for ob in range(2):
                    ps = pspwpool.tile([128, 4, 128], f32, tag="pw")
                    nc.tensor.matmul(
                        ps[:], lhsT=wpwt[0][ob][:],
                        rhs=xc0q[:, 4 * slot: 4 * slot + 4, :],
                        start=True, stop=False)
                    nc.tensor.matmul(
                        ps[:], lhsT=wpwt[1][ob][:],
                        rhs=xc1q[:, 4 * slot: 4 * slot + 4, :],
                        start=False, stop=True)
                    ys = yspool.tile([128, 4, 128], bf16, tag=f"ys{ob}")
                    if ch == "a":
                        nc.scalar.activation(out=ys[:], in_=ps[:],
                                             func=COPY)
                    else:
                        # Pool cannot access PSUM; 'g' falls back to DVE
                        nc.vector.tensor_copy(ys[:], ps[:])
                    nc.sync.dma_start(
                        out=y_ap[ob * 128:(ob + 1) * 128,
                                 4 * w: 4 * w + 4, :],
                        in_=ys[:])

            # ---------- main loop ----------
            for q in range(10):
                if q < 8:
                    s1_quad(q, 1, S1_PAT[2 * q])
                    s1_quad(q, 0, S1_PAT[2 * q + 1])
                if q >= 2:
                    bq = q - 2
                    xc0q = xc0pool.tile([128, 16, 128], bf16, tag="xc0")
                    xc1q = xc1pool.tile([128, 16, 128], bf16, tag="xc1")
                    # passthrough channels 0..63 via SBUF->SBUF DMA
                    nc.sync.dma_start(
                        out=xc0q[0:64, :, :],
                        in_=x0b0[0:64, 3 + 16 * bq: 3 + 16 * bq + 16,
                                 3: 131])
                    ch5 = BR5_PAT[bq]
                    for i in range(4):
                        if ch5 == "p":
                            br5_win_pe(4 * bq + i, xc1q, i)
                    ch3 = BR3_PAT[bq]
                    if ch3 == "p":
                        br3_quad_pe(bq, xc0q)
                    else:
                        br3_quad_hybrid(bq, xc0q, ch3)
                    for i in range(4):
                        pw_win(4 * bq + i, xc0q, xc1q, i, PWO_PAT[bq])
    return nc


def _prep_aux(w0, b0, w1, b1, w2, b2, w_pw, b_pw, bf16):
    d0 = np.zeros((2, 128, 9 * 128), dtype=bf16)
    for blk in range(2):
        for t, (ty, tx) in enumerate(T9):
            np.fill_diagonal(
                d0[blk, :, t * 128:(t + 1) * 128],
                w0[blk * 128:(blk + 1) * 128, 0, ty, tx].astype(bf16))
    d1 = np.zeros((128, 9 * 128), dtype=bf16)
    for t, (ty, tx) in enumerate(T9):
        vals = np.zeros(128, np.float32)
        vals[64:128] = w1[:, 0, ty, tx]
        np.fill_diagonal(d1[:, t * 128:(t + 1) * 128], vals.astype(bf16))
    d2 = np.zeros((128, 25 * 128), dtype=bf16)
    for t, (ty, tx) in enumerate(T25):
        vals = np.concatenate([w2[:, 0, ty, tx], w2[:, 0, ty, tx]])
        np.fill_diagonal(d2[:, t * 128:(t + 1) * 128], vals.astype(bf16))
    wpw = np.zeros((2, 2, 128, 128), dtype=bf16)
    for k in range(2):
        for ob in range(2):
            wpw[k, ob] = np.ascontiguousarray(
                w_pw[ob * 128:(ob + 1) * 128,
                     k * 128:(k + 1) * 128].T).astype(bf16)
    k0s = np.zeros((2, 128, 9), np.float32)
    for blk in range(2):
        for t, (ty, tx) in enumerate(T9):
            k0s[blk, :, t] = w0[blk * 128:(blk + 1) * 128, 0, ty, tx]
    k1s = np.zeros((128, 9), np.float32)
    for t, (ty, tx) in enumerate(T9):
        k1s[64:128, t] = w1[:, 0, ty, tx]
    b0v = b0.reshape(2, 128, 1).astype(np.float32)
    b1v = np.zeros((128, 1), np.float32)
    b1v[64:128, 0] = b1
    b2v = np.concatenate([b2, b2]).reshape(128, 1).astype(np.float32)
    return dict(d0=d0, d1=d1, d2=d2, wpw=wpw, k0s=k0s, k1s=k1s,
                b0v=b0v, b1v=b1v, b2v=b2v)


def kernel(x, w0, b0, w1, b1, w2, b2, w_pw, b_pw):
    import concourse.mybir as mybir
    from concourse.bass_utils import run_bass_kernel_spmd

    bf16 = mybir.dt.np(mybir.dt.bfloat16)

    if "nc" not in _CACHE:
        nc = _build_nc()
        _split_excess_waits(nc, mybir)
        _CACHE["nc"] = nc
    nc = _CACHE["nc"]

    x = np.asarray(x, np.float32)
    aux = _prep_aux(
        np.asarray(w0, np.float32), np.asarray(b0, np.float32),
        np.asarray(w1, np.float32), np.asarray(b1, np.float32),
        np.asarray(w2, np.float32), np.asarray(b2, np.float32),
        np.asarray(w_pw, np.float32), np.asarray(b_pw, np.float32),
        bf16)
    xpad = np.zeros((B, 2, 128, 130, 130), dtype=bf16)
    xpad[:, :, :, 1:129, 1:129] = x.reshape(B, 2, 128, 128, 128).astype(bf16)
    in_maps = [{"xb": xpad[i], **aux} for i in range(B)]
    res = run_bass_kernel_spmd(nc, in_maps, core_ids=list(range(B)))
    _CACHE["last_result"] = res
    out = np.stack([np.asarray(res.results[i]["y"]) for i in range(B)])
    return out.astype(np.float32) + np.asarray(
        b_pw, np.float32)[None, :, None, None]
